# revision 9
# baseline (speedup 1.0000x reference)
"""MultiHeadSelfAttention (B=2, S=2048, D=1024, H=16, causal) on 8 TRN2 NeuronCores.

Sharding: core c -> batch b = c//4, head-group g = c%4 (heads 4g..4g+3).
Each core: Q/K/V projections for its 4 heads over its batch's 2048 tokens,
causal softmax attention, partial output projection
out_partial = A_local @ WO[:, 256g:256g+256].T -> [2048, 1024].
Host sums the 4 partials per batch.

v4 redesign, driven by HW microbenchmarks (see mb.py):
- A [128,512] bf16 matmul whose stationary operand changes every issue
  costs ~375ns; with the same weights streamed twice (r=2 reuse) ~194ns;
  interleaving >2 PSUM accumulation groups degrades everything (~270+).
  Packed 64-row score pairs sustain ~87ns/MM; contiguous single-group
  65-col PV chains ~239ns/MM (vs ~390 interleaved per-kb).
- Q/K projections: weight-stationary r=2 -- each W chunk is streamed
  against two token quarters into two PSUM banks (kt-outer).
- PV: softmax weights pt are persisted per strip ([128,16,2,512] bf16),
  then each head's PV runs as ONE contiguous PSUM accumulation chain.
- Output projection: dt-outer r=2 (each merged chunk streams into both
  wo free halves).
- Emission: score pairs + exp are Act-gated (~1.1us/kb); "filler" units
  (projection passes, PV chains, tails, out-proj tiles) are interleaved
  between kb steps to keep the in-order PE queue dense.
- Engine balance: exp on Act; qt/kt drains + recips + normalize on DVE;
  V/oc drains + diag masks + odd-head shifts on GPSIMD(Pool); out-proj
  drains split Act/DVE.
"""
import numpy as np
import ml_dtypes

N_CORES = 8
B, S, D, H, DH = 2, 2048, 1024, 16, 64
HPG = 4            # heads per group (per core)
DL = HPG * DH      # 256 local features per core
NQB = S // 512     # 4 q-tiles of 512
NKB = S // 128     # 16 k-blocks of 128

_CACHE = {}


def _build(reps=1, delay_iters=0):
    import concourse.mybir as mybir
    import concourse.tile as tile
    from concourse import bacc
    from contextlib import ExitStack

    F32R = mybir.dt.float32r
    F32 = mybir.dt.float32
    BF16 = mybir.dt.bfloat16
    EXP = mybir.ActivationFunctionType.Exp

    nc = bacc.Bacc(trn_type="TRN2", target_bir_lowering=False, debug=False,
                   num_devices=N_CORES)

    xt_d = nc.dram_tensor("xt", [D, S], BF16, kind="ExternalInput").ap()
    wq_d = nc.dram_tensor("wq", [D, DL], BF16, kind="ExternalInput").ap()
    wk_d = nc.dram_tensor("wk", [D, DL], BF16, kind="ExternalInput").ap()
    wv_d = nc.dram_tensor("wv", [D, DL], BF16, kind="ExternalInput").ap()
    wo_d = nc.dram_tensor("wo", [DL, D], BF16, kind="ExternalInput").ap()
    mask_d = nc.dram_tensor("mask", [128, 1, 128], BF16, kind="ExternalInput").ap()
    ones2_d = nc.dram_tensor("ones2", [128, 128], F32R, kind="ExternalInput").ap()
    out_d = nc.dram_tensor("out", [S, D], BF16, kind="ExternalOutput").ap()

    with tile.TileContext(nc) as tc, ExitStack() as ctx:
        const = ctx.enter_context(tc.tile_pool(name="const", bufs=1))
        long_p = ctx.enter_context(tc.tile_pool(name="long", bufs=1))
        xt_p = ctx.enter_context(tc.tile_pool(name="xt", bufs=4))
        pt_p = ctx.enter_context(tc.tile_pool(name="pt", bufs=2))
        sc_p = ctx.enter_context(tc.tile_pool(name="scratch", bufs=4))
        outp = ctx.enter_context(tc.tile_pool(name="outp", bufs=4))
        ps_p = ctx.enter_context(tc.tile_pool(name="ps", bufs=2, space="PSUM"))
        st_p = ctx.enter_context(tc.tile_pool(name="st", bufs=1, space="PSUM"))
        ot_p = ctx.enter_context(tc.tile_pool(name="ot", bufs=2, space="PSUM"))

        # optional all-engine delay loop (timing calibration only)
        if delay_iters:
            with tc.For_i(0, delay_iters) as _i:
                with tc.tile_critical():
                    for _k in range(8):
                        nc.tensor.nop(cycle_cnt=60000)
                        nc.vector.nop(cycle_cnt=60000)
                        nc.scalar.nop(cycle_cnt=60000)
                        nc.gpsimd.nop(cycle_cnt=60000)
                        nc.sync.nop(cycle_cnt=60000)

        wqr = wq_d.rearrange("(c p) d -> p c d", p=128)
        wkr = wk_d.rearrange("(c p) d -> p c d", p=128)
        wvr = wv_d.rearrange("(c p) d -> p c d", p=128)
        wor = wo_d.rearrange("(c p) d -> p c d", p=128)
        xtr = xt_d.rearrange("(c p) s -> p c s", p=128)
        w_tiles = []
        for name in ("wq", "wk", "wv"):
            wt = const.tile([128, 8, DL], BF16, tag=name, name=name)
            w_tiles.append(wt)
        w_tiles = {"wq": w_tiles[0], "wk": w_tiles[1], "wv": w_tiles[2]}
        xts = [xt_p.tile([128, 8, 512], BF16, tag="xt", name=f"xts{q}")
               for q in range(4)]
        # startup DMAs ordered by consumption: V(q0) runs first (needs wv +
        # xt q0), then Q/K r=2 passes need q0+q1.
        nc.sync.dma_start(w_tiles["wv"][:, 0:4, :], wvr[:, 0:4, :])
        nc.sync.dma_start(xts[0][:, 0, :], xtr[:, 0, 0:512])
        nc.sync.dma_start(w_tiles["wv"][:, 4:8, :], wvr[:, 4:8, :])
        for kt in range(1, 8):
            nc.sync.dma_start(xts[0][:, kt, :], xtr[:, kt, 0:512])
        nc.sync.dma_start(w_tiles["wq"][:, 0:8, :], wqr[:, 0:8, :])
        nc.sync.dma_start(xts[1][:, 0:4, :], xtr[:, 0:4, 512:1024])
        nc.sync.dma_start(xts[1][:, 4:8, :], xtr[:, 4:8, 512:1024])
        nc.sync.dma_start(w_tiles["wk"][:, 0:8, :], wkr[:, 0:8, :])
        for q in range(2, 4):
            nc.sync.dma_start(xts[q][:, 0:4, :],
                              xtr[:, 0:4, q * 512:(q + 1) * 512])
            nc.sync.dma_start(xts[q][:, 4:8, :],
                              xtr[:, 4:8, q * 512:(q + 1) * 512])
        ones2_t = const.tile([128, 128], F32R)
        nc.sync.dma_start(ones2_t[:], ones2_d)
        mask_t = const.tile([128, 1, 128], BF16)
        nc.sync.dma_start(mask_t[:], mask_d)
        wo_t = const.tile([128, 2, D], BF16)
        nc.sync.dma_start(wo_t[:], wor[:])

        for _rep in range(reps):
            oc_q = [[None] * 4 for _ in range(4)]
            zr_q = [[None] * 4 for _ in range(4)]
            pt_s = {}          # (qb, ft) -> persisted softmax-weight tile
            qt_q, kt_q, vaug_q, merged_q = [], [], [], []
            for q in range(4):
                qt = long_p.tile([128, 2, 512], BF16, tag=f"qt{q}")
                kt_ = long_p.tile([128, 2, 512], BF16, tag=f"kt{q}")
                va = long_p.tile([128, 4, HPG, DH + 1], BF16, tag=f"va{q}")
                mg = long_p.tile([128, 2, 512], BF16, tag=f"mg{q}")
                qt_q.append(qt)
                kt_q.append(kt_)
                vaug_q.append(va)
                merged_q.append(mg)

            if _rep == 0:
                xt_use = xts
            else:
                xt_use = [xt_p.tile([128, 8, 512], BF16, tag="xt",
                                    name=f"xtq{q}") for q in range(4)]
                for q in range(4):
                    nc.sync.dma_start(xt_use[q][:, 0:4, :],
                                      xtr[:, 0:4, q * 512:(q + 1) * 512])
                    nc.sync.dma_start(xt_use[q][:, 4:8, :],
                                      xtr[:, 4:8, q * 512:(q + 1) * 512])

            # ---------- emission units ----------
            def unit_vones(q):
                nc.gpsimd.tensor_copy(
                    vaug_q[q][:, :, :, DH:DH + 1],
                    ones2_t[:, 0:16].rearrange("p (a b) -> p a b", a=4))

            def unit_v(q, tl):
                # V token-major: contiguous chain over kt, drain on GPSIMD
                vp = ps_p.tile([128, 256], F32, tag="ps", name="vp")
                for kt in range(8):
                    nc.tensor.matmul(
                        vp[:], xt_use[q][:, kt, tl * 128:(tl + 1) * 128],
                        w_tiles["wv"][:, kt, :], start=(kt == 0), stop=(kt == 7))
                nc.vector.tensor_copy(
                    vaug_q[q][:, tl, :, 0:DH],
                    vp[:].rearrange("p (h d) -> p h d", h=HPG))

            def unit_qk(pi, ft, qa, qb):
                # weight-stationary r=2: W chunk streams against quarters
                # qa and qb into two PSUM banks; drains on DVE
                psa = ps_p.tile([128, 512], F32, tag="ps", name="psa")
                psb = ps_p.tile([128, 512], F32, tag="ps", name="psb")
                for kt in range(8):
                    w_ap = w_tiles[pi][:, kt, ft * 128:(ft + 1) * 128]
                    nc.tensor.matmul(psa[:], w_ap, xt_use[qa][:, kt, :],
                                     start=(kt == 0), stop=(kt == 7))
                    nc.tensor.matmul(psb[:], w_ap, xt_use[qb][:, kt, :],
                                     start=(kt == 0), stop=(kt == 7))
                dest = qt_q if pi == "wq" else kt_q
                nc.vector.tensor_copy(dest[qa][:, ft, :], psa[:])
                nc.vector.tensor_copy(dest[qb][:, ft, :], psb[:])

            st_t = st_p.tile([128, 4, 512], F32, tag="st", name="st_t")

            def emit_score_kb(qb, ft, kb):
                # one k-block: packed 64-row score pair into slot (kb%2);
                # full-block pairs share ONE exp (halves Act instr count),
                # diag blocks exp per-kb (avoids stale-region exp)
                pt = pt_s[(qb, ft)]
                kq, tl = kb // 4, kb % 4
                r = kb - 4 * qb
                off = 128 * r if r > 0 else 0
                m = kb % 2
                for hp in range(2):
                    nc.tensor.matmul(
                        st_t[:, 2 * m + hp, off:],
                        kt_q[kq][hp * 64:hp * 64 + 64, ft,
                                 tl * 128:(tl + 1) * 128],
                        qt_q[qb][hp * 64:hp * 64 + 64, ft, off:],
                        start=True, stop=True,
                        tile_position=(hp * 64, 0))
                if r < 0:
                    if m == 1:
                        nc.scalar.activation(
                            pt[:, kb - 1:kb + 1, :, :],
                            st_t[:, :, :].rearrange("p (k h) f -> p k h f", k=2),
                            EXP, scale=0.125)
                else:
                    nc.scalar.activation(pt[:, kb, :, off:],
                                         st_t[:, 2 * m:2 * m + 2, off:],
                                         EXP, scale=0.125)
                    nc.gpsimd.tensor_mul(
                        pt[:, kb, :, off:off + 128], pt[:, kb, :, off:off + 128],
                        mask_t[:, 0:1, :].broadcast_to([128, 2, 128]))

            def unit_pv(qb, ft, hp):
                # ONE contiguous 65-col accumulation chain per head
                pt = pt_s[(qb, ft)]
                nkb = 4 * (qb + 1)
                ot = ot_p.tile([DH + 1, 512], F32, tag="ot", name="ot")
                for kb in range(nkb):
                    kq, tl = kb // 4, kb % 4
                    r = kb - 4 * qb
                    off = 128 * r if r > 0 else 0
                    nc.tensor.matmul(
                        ot[:, off:],
                        vaug_q[kq][:, tl, 2 * ft + hp, :],
                        pt[:, kb, hp, off:],
                        start=(kb == 0), stop=(kb == nkb - 1))
                # numerator rows to SBUF promptly: frees the bank; GPSIMD
                # does the bulk copy, DVE the reciprocal
                zr = sc_p.tile([DH + 1, 512], F32R, tag="zr", name="zr")
                with nc.allow_low_precision(reason="softmax denom recip"):
                    nc.vector.reciprocal(zr[DH:DH + 1, :], ot[DH:DH + 1, :])
                oc = sc_p.tile([64, 512], F32R, tag="oc", name="oc")
                if hp == 0:
                    nc.vector.tensor_copy(oc[:], ot[0:64, :])
                else:
                    nc.scalar.copy(oc[:], ot[0:64, :])
                oc_q[qb][2 * ft + hp] = oc
                zr_q[qb][2 * ft + hp] = zr

            def unit_tail(q, ft, last=False):
                # normalization: PE broadcast of 1/Z, DVE multiply,
                # odd-head partition shift via GPSIMD SWDGE
                for hp in range(2):
                    oc = oc_q[q][2 * ft + hp]
                    zr = zr_q[q][2 * ft + hp]
                    bc = ps_p.tile([64, 512], F32, tag="ps", name="bc")
                    nc.tensor.matmul(bc[:], ones2_t[64:65, 0:64],
                                     zr[DH:DH + 1, :], start=True, stop=True)
                    if hp == 0:
                        nc.vector.tensor_mul(merged_q[q][0:64, ft, :],
                                             oc[:], bc[:])
                    else:
                        odd = sc_p.tile([64, 512], BF16, tag="odd", name="odd")
                        nc.vector.tensor_mul(odd[:], oc[:], bc[:])
                        if last:
                            nc.sync.dma_start(merged_q[q][64:128, ft, :],
                                              odd[:])
                        else:
                            nc.gpsimd.dma_start(merged_q[q][64:128, ft, :],
                                                odd[:])

            def unit_outproj(q, tl):
                # 64-col one-shot token-half pairs per dt chunk (LDW fully
                # hidden); the dt sum is folded into the DVE drain add
                ts = 4 * q + tl
                osb = outp.tile([128, 1024], BF16, tag="osb", name="osb")
                for fs in range(2):
                    pd0 = ps_p.tile([128, 512], F32, tag="ps", name="pd0")
                    pd1 = ps_p.tile([128, 512], F32, tag="ps", name="pd1")
                    for dt_i, pd in ((0, pd0), (1, pd1)):
                        for hf in range(2):
                            nc.tensor.matmul(
                                pd[hf * 64:(hf + 1) * 64, :],
                                merged_q[q][:, dt_i,
                                            tl * 128 + hf * 64:
                                            tl * 128 + (hf + 1) * 64],
                                wo_t[:, dt_i, fs * 512:(fs + 1) * 512],
                                start=True, stop=True,
                                tile_position=(0, hf * 64))
                    # DVE may read only ONE PSUM operand per instruction:
                    # copy dt0, then in-place add dt1
                    nc.vector.tensor_copy(osb[:, fs * 512:(fs + 1) * 512],
                                          pd0[:])
                    nc.vector.tensor_add(osb[:, fs * 512:(fs + 1) * 512],
                                         osb[:, fs * 512:(fs + 1) * 512],
                                         pd1[:])
                nc.sync.dma_start(out_d[ts * 128:(ts + 1) * 128, :], osb[:])

            # ---------- schedule ----------
            def scores_loop(qb, ft, fillers):
                pt_s[(qb, ft)] = pt_p.tile([128, 16, 2, 512], BF16, tag="pt",
                                           name=f"pt{qb}{ft}")
                nkb = 4 * (qb + 1)
                fi = 0
                for kb in range(nkb):
                    emit_score_kb(qb, ft, kb)
                    if fi < len(fillers):
                        fillers[fi]()
                        fi += 1
                while fi < len(fillers):
                    fillers[fi]()
                    fi += 1

            unit_vones(0)
            for tl in range(4):
                unit_v(0, tl)
            unit_qk("wq", 0, 0, 1)
            unit_qk("wk", 0, 0, 1)
            unit_vones(1)
            unit_vones(2)
            unit_vones(3)

            scores_loop(0, 0, [
                lambda: unit_qk("wq", 1, 0, 1),
                lambda: unit_qk("wk", 1, 0, 1),
                lambda: unit_v(1, 0),
                lambda: unit_v(1, 1),
            ])
            scores_loop(0, 1, [
                lambda: unit_v(1, 2),
                lambda: unit_v(1, 3),
                lambda: unit_pv(0, 0, 0),
                lambda: unit_pv(0, 0, 1),
            ])
            scores_loop(1, 0, [
                lambda: unit_qk("wq", 0, 2, 3),
                lambda: unit_pv(0, 1, 0),
                lambda: unit_pv(0, 1, 1),
                lambda: unit_qk("wk", 0, 2, 3),
                lambda: unit_v(2, 0),
                lambda: unit_v(2, 1),
                lambda: unit_tail(0, 0),
                lambda: unit_tail(0, 1),
            ])
            scores_loop(1, 1, [
                lambda: unit_qk("wq", 1, 2, 3),
                lambda: unit_qk("wk", 1, 2, 3),
                lambda: unit_pv(1, 0, 0),
                lambda: unit_pv(1, 0, 1),
                lambda: unit_v(2, 2),
                lambda: unit_v(2, 3),
                lambda: unit_outproj(0, 0),
                lambda: unit_outproj(0, 1),
            ])
            scores_loop(2, 0, [
                lambda: unit_pv(1, 1, 0),
                lambda: unit_pv(1, 1, 1),
                lambda: unit_v(3, 0),
                lambda: unit_v(3, 1),
                lambda: unit_tail(1, 0),
                lambda: unit_tail(1, 1),
                lambda: unit_outproj(0, 2),
                lambda: unit_outproj(0, 3),
                lambda: unit_v(3, 2),
                lambda: unit_v(3, 3),
            ])
            scores_loop(2, 1, [
                lambda: unit_pv(2, 0, 0),
                lambda: unit_pv(2, 0, 1),
                lambda: unit_outproj(1, 0),
                lambda: unit_outproj(1, 1),
                lambda: unit_outproj(1, 2),
                lambda: unit_outproj(1, 3),
            ])
            scores_loop(3, 0, [
                lambda: unit_pv(2, 1, 0),
                lambda: unit_pv(2, 1, 1),
                lambda: unit_tail(2, 0),
                lambda: unit_tail(2, 1),
                lambda: unit_outproj(2, 0),
                lambda: unit_outproj(2, 1),
                lambda: unit_outproj(2, 2),
                lambda: unit_outproj(2, 3),
            ])
            scores_loop(3, 1, [
                lambda: unit_pv(3, 0, 0),
                lambda: unit_pv(3, 0, 1),
                lambda: unit_tail(3, 0),
            ])
            unit_pv(3, 1, 0)
            unit_pv(3, 1, 1)
            unit_tail(3, 1, last=True)
            for tl in range(4):
                unit_outproj(3, tl)
    nc.compile()
    return nc


def _make_in_maps(x, WQ, WK, WV, WO):
    i = np.arange(128)[:, None]
    j = np.arange(128)[None, :]
    mask = (j >= i).astype(ml_dtypes.bfloat16).reshape(128, 1, 128)
    ones2 = np.ones((128, 128), dtype=np.float32)
    in_maps = []
    for c in range(N_CORES):
        b, g = c // HPG, c % HPG
        sl = slice(DL * g, DL * (g + 1))
        in_maps.append({
            "xt": np.ascontiguousarray(x[b].T).astype(ml_dtypes.bfloat16),
            "wq": np.ascontiguousarray(WQ[sl, :].T).astype(ml_dtypes.bfloat16),
            "wk": np.ascontiguousarray(WK[sl, :].T).astype(ml_dtypes.bfloat16),
            "wv": np.ascontiguousarray(WV[sl, :].T).astype(ml_dtypes.bfloat16),
            "wo": np.ascontiguousarray(WO[:, sl].T).astype(ml_dtypes.bfloat16),
            "mask": mask, "ones2": ones2,
        })
    return in_maps


def get_compiled(reps=1, delay_iters=0):
    key = (reps, delay_iters)
    if key not in _CACHE:
        _CACHE[key] = _build(reps, delay_iters)
    return _CACHE[key]


def kernel(x, WQ, WK, WV, WO):
    from concourse import bass_utils
    x = np.asarray(x, dtype=np.float32)
    WQ = np.asarray(WQ, dtype=np.float32)
    WK = np.asarray(WK, dtype=np.float32)
    WV = np.asarray(WV, dtype=np.float32)
    WO = np.asarray(WO, dtype=np.float32)
    nc = get_compiled()
    in_maps = _make_in_maps(x, WQ, WK, WV, WO)
    res = bass_utils.run_bass_kernel_spmd(nc, in_maps, core_ids=list(range(N_CORES)))
    out = np.zeros((B, S, D), dtype=np.float32)
    for c in range(N_CORES):
        out[c // HPG] += res.results[c]["out"].astype(np.float32)
    return out


# revision 14
# speedup vs baseline: 1.3783x; 1.3783x over previous
"""MultiHeadSelfAttention (B=2, S=2048, D=1024, H=16, causal) on 8 TRN2 NeuronCores.

Sharding: core c -> batch b = c//4, head-group g = c%4 (heads 4g..4g+3).
Each core: Q/K/V projections for its 4 heads over its batch's 2048 tokens,
causal softmax attention, partial output projection
out_partial = A_local @ WO[:, 256g:256g+256].T -> [2048, 1024].
Host sums the 4 partials per batch.

v4 redesign, driven by HW microbenchmarks (see mb.py):
- A [128,512] bf16 matmul whose stationary operand changes every issue
  costs ~375ns; with the same weights streamed twice (r=2 reuse) ~194ns;
  interleaving >2 PSUM accumulation groups degrades everything (~270+).
  Packed 64-row score pairs sustain ~87ns/MM; contiguous single-group
  65-col PV chains ~239ns/MM (vs ~390 interleaved per-kb).
- Q/K projections: weight-stationary r=2 -- each W chunk is streamed
  against two token quarters into two PSUM banks (kt-outer).
- PV: softmax weights pt are persisted per strip ([128,16,2,512] bf16),
  then each head's PV runs as ONE contiguous PSUM accumulation chain.
- Output projection: dt-outer r=2 (each merged chunk streams into both
  wo free halves).
- Emission: score pairs + exp are Act-gated (~1.1us/kb); "filler" units
  (projection passes, PV chains, tails, out-proj tiles) are interleaved
  between kb steps to keep the in-order PE queue dense.
- Engine balance: exp on Act; qt/kt drains + recips + normalize on DVE;
  V/oc drains + diag masks + odd-head shifts on GPSIMD(Pool); out-proj
  drains split Act/DVE.
"""
import numpy as np
import ml_dtypes

N_CORES = 8
B, S, D, H, DH = 2, 2048, 1024, 16, 64
HPG = 4            # heads per group (per core)
DL = HPG * DH      # 256 local features per core
NQB = S // 512     # 4 q-tiles of 512
NKB = S // 128     # 16 k-blocks of 128

_CACHE = {}


def _build(reps=1, delay_iters=0):
    import concourse.mybir as mybir
    import concourse.tile as tile
    from concourse import bacc
    from contextlib import ExitStack

    F32R = mybir.dt.float32r
    F32 = mybir.dt.float32
    BF16 = mybir.dt.bfloat16
    EXP = mybir.ActivationFunctionType.Exp

    nc = bacc.Bacc(trn_type="TRN2", target_bir_lowering=False, debug=False,
                   num_devices=N_CORES)

    xt_d = nc.dram_tensor("xt", [D, S], BF16, kind="ExternalInput").ap()
    wq_d = nc.dram_tensor("wq", [D, DL], BF16, kind="ExternalInput").ap()
    wk_d = nc.dram_tensor("wk", [D, DL], BF16, kind="ExternalInput").ap()
    wv_d = nc.dram_tensor("wv", [D, DL], BF16, kind="ExternalInput").ap()
    wo_d = nc.dram_tensor("wo", [DL, D], BF16, kind="ExternalInput").ap()
    mask_d = nc.dram_tensor("mask", [128, 1, 128], BF16, kind="ExternalInput").ap()
    ones2_d = nc.dram_tensor("ones2", [128, 128], F32R, kind="ExternalInput").ap()
    out_d = nc.dram_tensor("out", [S, D], BF16, kind="ExternalOutput").ap()

    with tile.TileContext(nc) as tc, ExitStack() as ctx:
        const = ctx.enter_context(tc.tile_pool(name="const", bufs=1))
        long_p = ctx.enter_context(tc.tile_pool(name="long", bufs=1))
        xt_p = ctx.enter_context(tc.tile_pool(name="xt", bufs=4))
        pt_p = ctx.enter_context(tc.tile_pool(name="pt", bufs=2))
        sc_p = ctx.enter_context(tc.tile_pool(name="scratch", bufs=4))
        outp = ctx.enter_context(tc.tile_pool(name="outp", bufs=4))
        ps_p = ctx.enter_context(tc.tile_pool(name="ps", bufs=2, space="PSUM"))
        st_p = ctx.enter_context(tc.tile_pool(name="st", bufs=2, space="PSUM"))
        ot_p = ctx.enter_context(tc.tile_pool(name="ot", bufs=2, space="PSUM"))

        # optional all-engine delay loop (timing calibration only)
        if delay_iters:
            with tc.For_i(0, delay_iters) as _i:
                with tc.tile_critical():
                    for _k in range(8):
                        nc.tensor.nop(cycle_cnt=60000)
                        nc.vector.nop(cycle_cnt=60000)
                        nc.scalar.nop(cycle_cnt=60000)
                        nc.gpsimd.nop(cycle_cnt=60000)
                        nc.sync.nop(cycle_cnt=60000)

        wqr = wq_d.rearrange("(c p) d -> p c d", p=128)
        wkr = wk_d.rearrange("(c p) d -> p c d", p=128)
        wvr = wv_d.rearrange("(c p) d -> p c d", p=128)
        wor = wo_d.rearrange("(c p) d -> p c d", p=128)
        xtr = xt_d.rearrange("(c p) s -> p c s", p=128)
        w_tiles = []
        for name in ("wq", "wk", "wv"):
            wt = const.tile([128, 8, DL], BF16, tag=name, name=name)
            w_tiles.append(wt)
        w_tiles = {"wq": w_tiles[0], "wk": w_tiles[1], "wv": w_tiles[2]}
        xts = [xt_p.tile([128, 8, 512], BF16, tag="xt", name=f"xts{q}")
               for q in range(4)]
        # startup DMAs ordered by consumption: V(q0) runs first (needs wv +
        # xt q0), then Q/K r=2 passes need q0+q1.
        nc.sync.dma_start(w_tiles["wv"][:, 0:4, :], wvr[:, 0:4, :])
        nc.sync.dma_start(xts[0][:, 0, :], xtr[:, 0, 0:512])
        nc.sync.dma_start(w_tiles["wv"][:, 4:8, :], wvr[:, 4:8, :])
        for kt in range(1, 8):
            nc.sync.dma_start(xts[0][:, kt, :], xtr[:, kt, 0:512])
        nc.sync.dma_start(w_tiles["wq"][:, 0:8, :], wqr[:, 0:8, :])
        nc.sync.dma_start(xts[1][:, 0:4, :], xtr[:, 0:4, 512:1024])
        nc.sync.dma_start(xts[1][:, 4:8, :], xtr[:, 4:8, 512:1024])
        nc.sync.dma_start(w_tiles["wk"][:, 0:8, :], wkr[:, 0:8, :])
        for q in range(2, 4):
            nc.sync.dma_start(xts[q][:, 0:4, :],
                              xtr[:, 0:4, q * 512:(q + 1) * 512])
            nc.sync.dma_start(xts[q][:, 4:8, :],
                              xtr[:, 4:8, q * 512:(q + 1) * 512])
        ones2_t = const.tile([128, 128], F32R)
        nc.sync.dma_start(ones2_t[:], ones2_d)
        mask_t = const.tile([128, 1, 128], BF16)
        nc.sync.dma_start(mask_t[:], mask_d)
        wo_t = const.tile([128, 2, D], BF16)
        nc.sync.dma_start(wo_t[:], wor[:])

        for _rep in range(reps):
            oc_q = [[None] * 4 for _ in range(4)]
            zr_q = [[None] * 4 for _ in range(4)]
            pt_s = {}          # (qb, ft) -> persisted softmax-weight tile
            qt_q, kt_q, vaug_q, merged_q = [], [], [], []
            for q in range(4):
                qt = long_p.tile([128, 2, 512], BF16, tag=f"qt{q}")
                kt_ = long_p.tile([128, 2, 512], BF16, tag=f"kt{q}")
                va = long_p.tile([128, 4, HPG, DH + 1], BF16, tag=f"va{q}")
                mg = long_p.tile([128, 2, 512], BF16, tag=f"mg{q}")
                qt_q.append(qt)
                kt_q.append(kt_)
                vaug_q.append(va)
                merged_q.append(mg)

            if _rep == 0:
                xt_use = xts
            else:
                xt_use = [xt_p.tile([128, 8, 512], BF16, tag="xt",
                                    name=f"xtq{q}") for q in range(4)]
                for q in range(4):
                    nc.sync.dma_start(xt_use[q][:, 0:4, :],
                                      xtr[:, 0:4, q * 512:(q + 1) * 512])
                    nc.sync.dma_start(xt_use[q][:, 4:8, :],
                                      xtr[:, 4:8, q * 512:(q + 1) * 512])

            # ---------- emission units ----------
            def unit_vones(q):
                nc.gpsimd.tensor_copy(
                    vaug_q[q][:, :, :, DH:DH + 1],
                    ones2_t[:, 0:16].rearrange("p (a b) -> p a b", a=4))

            def unit_v(q, tl):
                # V token-major: contiguous chain over kt, drain on GPSIMD
                vp = ps_p.tile([128, 256], F32, tag="ps", name="vp")
                for kt in range(8):
                    nc.tensor.matmul(
                        vp[:], xt_use[q][:, kt, tl * 128:(tl + 1) * 128],
                        w_tiles["wv"][:, kt, :], start=(kt == 0), stop=(kt == 7))
                nc.vector.tensor_copy(
                    vaug_q[q][:, tl, :, 0:DH],
                    vp[:].rearrange("p (h d) -> p h d", h=HPG))

            def unit_qk(pi, ft, qa, qb):
                # weight-stationary r=2: W chunk streams against quarters
                # qa and qb into two PSUM banks; drains on DVE
                psa = ps_p.tile([128, 512], F32, tag="ps", name="psa")
                psb = ps_p.tile([128, 512], F32, tag="ps", name="psb")
                for kt in range(8):
                    w_ap = w_tiles[pi][:, kt, ft * 128:(ft + 1) * 128]
                    nc.tensor.matmul(psa[:], w_ap, xt_use[qa][:, kt, :],
                                     start=(kt == 0), stop=(kt == 7))
                    nc.tensor.matmul(psb[:], w_ap, xt_use[qb][:, kt, :],
                                     start=(kt == 0), stop=(kt == 7))
                dest = qt_q if pi == "wq" else kt_q
                nc.vector.tensor_copy(dest[qa][:, ft, :], psa[:])
                nc.vector.tensor_copy(dest[qb][:, ft, :], psb[:])

            def emit_score_kb(qb, ft, kb):
                # one k-block: packed 64-row score pair -> exp -> diag mask
                pt = pt_s[(qb, ft)]
                kq, tl = kb // 4, kb % 4
                r = kb - 4 * qb
                off = 128 * r if r > 0 else 0
                st = st_p.tile([128, 2, 512], F32, tag="st", name="st")
                for hp in range(2):
                    nc.tensor.matmul(
                        st[:, hp, off:],
                        kt_q[kq][hp * 64:hp * 64 + 64, ft,
                                 tl * 128:(tl + 1) * 128],
                        qt_q[qb][hp * 64:hp * 64 + 64, ft, off:],
                        start=True, stop=True,
                        tile_position=(hp * 64, 0))
                nc.scalar.activation(pt[:, kb, :, off:], st[:, :, off:],
                                     EXP, scale=0.125)
                if r >= 0:
                    nc.gpsimd.tensor_mul(
                        pt[:, kb, :, off:off + 128], pt[:, kb, :, off:off + 128],
                        mask_t[:, 0:1, :].broadcast_to([128, 2, 128]))

            def pv_units(qb, ft, hp, seg=5):
                # 65-col accumulation chain per head, split into <=seg-kb
                # filler segments (ot group stays open across; only one-shot
                # score pairs run between, which doesn't break chain rate)
                nkb = 4 * (qb + 1)
                state = {}

                def make(lo, hi):
                    def run():
                        pt = pt_s[(qb, ft)]
                        if lo == 0:
                            state["ot"] = ot_p.tile([DH + 1, 512], F32,
                                                    tag="ot", name="ot")
                        ot = state["ot"]
                        for kb in range(lo, hi):
                            kq, tl = kb // 4, kb % 4
                            r = kb - 4 * qb
                            off = 128 * r if r > 0 else 0
                            nc.tensor.matmul(
                                ot[:, off:],
                                vaug_q[kq][:, tl, 2 * ft + hp, :],
                                pt[:, kb, hp, off:],
                                start=(kb == 0), stop=(kb == nkb - 1))
                        if hi == nkb:
                            # numerator rows to SBUF promptly (frees bank)
                            zr = sc_p.tile([DH + 1, 512], F32R, tag="zr",
                                           name="zr")
                            with nc.allow_low_precision(
                                    reason="softmax denom recip"):
                                nc.vector.reciprocal(zr[DH:DH + 1, :],
                                                     ot[DH:DH + 1, :])
                            oc = sc_p.tile([64, 512], F32R, tag="oc",
                                           name="oc")
                            if hp == 0:
                                nc.vector.tensor_copy(oc[:], ot[0:64, :])
                            else:
                                nc.scalar.copy(oc[:], ot[0:64, :])
                            oc_q[qb][2 * ft + hp] = oc
                            zr_q[qb][2 * ft + hp] = zr
                    return run

                return [make(lo, min(lo + seg, nkb))
                        for lo in range(0, nkb, seg)]

            def unit_pv(qb, ft, hp):
                for u in pv_units(qb, ft, hp, seg=64):
                    u()

            def unit_tail(q, ft, last=False):
                # normalization: PE broadcast of 1/Z, DVE multiply,
                # odd-head partition shift via GPSIMD SWDGE
                for hp in range(2):
                    oc = oc_q[q][2 * ft + hp]
                    zr = zr_q[q][2 * ft + hp]
                    bc = ps_p.tile([64, 512], F32, tag="ps", name="bc")
                    nc.tensor.matmul(bc[:], ones2_t[64:65, 0:64],
                                     zr[DH:DH + 1, :], start=True, stop=True)
                    if hp == 0:
                        nc.vector.tensor_mul(merged_q[q][0:64, ft, :],
                                             oc[:], bc[:])
                    else:
                        odd = sc_p.tile([64, 512], BF16, tag="odd", name="odd")
                        nc.vector.tensor_mul(odd[:], oc[:], bc[:])
                        if last:
                            nc.sync.dma_start(merged_q[q][64:128, ft, :],
                                              odd[:])
                        else:
                            nc.gpsimd.dma_start(merged_q[q][64:128, ft, :],
                                                odd[:])

            def unit_outproj(q, tl):
                # dt-outer r=2: each merged chunk streams into both wo
                # free halves; drains split Act/DVE
                ts = 4 * q + tl
                osb = outp.tile([128, 1024], BF16, tag="osb", name="osb")
                ops0 = ps_p.tile([128, 512], F32, tag="ps", name="ops0")
                ops1 = ps_p.tile([128, 512], F32, tag="ps", name="ops1")
                for dt_i in range(2):
                    mg_ap = merged_q[q][:, dt_i, tl * 128:(tl + 1) * 128]
                    nc.tensor.matmul(ops0[:], mg_ap, wo_t[:, dt_i, 0:512],
                                     start=(dt_i == 0), stop=(dt_i == 1))
                    nc.tensor.matmul(ops1[:], mg_ap, wo_t[:, dt_i, 512:1024],
                                     start=(dt_i == 0), stop=(dt_i == 1))
                nc.scalar.copy(osb[:, 0:512], ops0[:])
                nc.vector.tensor_copy(osb[:, 512:1024], ops1[:])
                nc.sync.dma_start(out_d[ts * 128:(ts + 1) * 128, :], osb[:])

            # ---------- schedule ----------
            def scores_loop(qb, ft, fillers):
                pt_s[(qb, ft)] = pt_p.tile([128, 16, 2, 512], BF16, tag="pt",
                                           name=f"pt{qb}{ft}")
                nkb = 4 * (qb + 1)
                fi = 0
                for kb in range(nkb):
                    emit_score_kb(qb, ft, kb)
                    if fi < len(fillers):
                        fillers[fi]()
                        fi += 1
                while fi < len(fillers):
                    fillers[fi]()
                    fi += 1

            unit_vones(0)
            for tl in range(4):
                unit_v(0, tl)
            unit_qk("wq", 0, 0, 1)
            unit_qk("wk", 0, 0, 1)
            unit_vones(1)
            unit_vones(2)
            unit_vones(3)

            # pt slot rotation (bufs=2): A,B,A,B,... in loop order below.
            # RULE: PV(X) units must be fully emitted in loops BEFORE the
            # loop that reuses X's pt slot (2 loops later), else the in-order
            # PE queue deadlocks on the st WAR -> exp -> pt WAR chain.
            # qb=3 strips run BEFORE qb=2 so the post-loop tail is shorter.
            scores_loop(0, 0, [
                lambda: unit_qk("wq", 1, 0, 1),
                lambda: unit_qk("wk", 1, 0, 1),
                lambda: unit_v(1, 0),
                lambda: unit_v(1, 1),
            ])
            scores_loop(0, 1,
                pv_units(0, 0, 0) + pv_units(0, 0, 1) + [
                lambda: unit_v(1, 2),
                lambda: unit_v(1, 3),
            ])
            scores_loop(1, 0,
                pv_units(0, 1, 0) + pv_units(0, 1, 1) + [
                lambda: unit_qk("wq", 0, 2, 3),
                lambda: unit_qk("wk", 0, 2, 3),
                lambda: unit_tail(0, 0),
                lambda: unit_tail(0, 1),
            ])
            scores_loop(1, 1,
                pv_units(1, 0, 0) + pv_units(1, 0, 1) + [
                lambda: unit_qk("wq", 1, 2, 3),
                lambda: unit_qk("wk", 1, 2, 3),
                lambda: unit_v(2, 0),
                lambda: unit_v(2, 1),
            ])
            scores_loop(3, 0,
                pv_units(1, 1, 0) + pv_units(1, 1, 1) + [
                lambda: unit_tail(1, 0),
                lambda: unit_tail(1, 1),
                lambda: unit_v(2, 2),
                lambda: unit_v(2, 3),
                lambda: unit_v(3, 0),
                lambda: unit_v(3, 1),
                lambda: unit_v(3, 2),
                lambda: unit_v(3, 3),
                lambda: unit_outproj(0, 0),
                lambda: unit_outproj(0, 1),
                lambda: unit_outproj(0, 2),
                lambda: unit_outproj(0, 3),
            ])
            scores_loop(3, 1,
                pv_units(3, 0, 0) + pv_units(3, 0, 1) + [
                lambda: unit_tail(3, 0),
                lambda: unit_outproj(1, 0),
                lambda: unit_outproj(1, 1),
                lambda: unit_outproj(1, 2),
                lambda: unit_outproj(1, 3),
            ])
            scores_loop(2, 0,
                pv_units(3, 1, 0) + pv_units(3, 1, 1) + [
                lambda: unit_tail(3, 1),
                lambda: unit_outproj(3, 0),
                lambda: unit_outproj(3, 1),
                lambda: unit_outproj(3, 2),
                lambda: unit_outproj(3, 3),
            ])
            scores_loop(2, 1,
                pv_units(2, 0, 0) + pv_units(2, 0, 1) + [
                lambda: unit_tail(2, 0),
            ])
            unit_pv(2, 1, 0)
            unit_pv(2, 1, 1)
            unit_tail(2, 1, last=True)
            for tl in range(4):
                unit_outproj(2, tl)
    nc.compile()
    return nc


def _make_in_maps(x, WQ, WK, WV, WO):
    i = np.arange(128)[:, None]
    j = np.arange(128)[None, :]
    mask = (j >= i).astype(ml_dtypes.bfloat16).reshape(128, 1, 128)
    ones2 = np.ones((128, 128), dtype=np.float32)
    in_maps = []
    for c in range(N_CORES):
        b, g = c // HPG, c % HPG
        sl = slice(DL * g, DL * (g + 1))
        in_maps.append({
            "xt": np.ascontiguousarray(x[b].T).astype(ml_dtypes.bfloat16),
            "wq": np.ascontiguousarray(WQ[sl, :].T).astype(ml_dtypes.bfloat16),
            "wk": np.ascontiguousarray(WK[sl, :].T).astype(ml_dtypes.bfloat16),
            "wv": np.ascontiguousarray(WV[sl, :].T).astype(ml_dtypes.bfloat16),
            "wo": np.ascontiguousarray(WO[:, sl].T).astype(ml_dtypes.bfloat16),
            "mask": mask, "ones2": ones2,
        })
    return in_maps


def get_compiled(reps=1, delay_iters=0):
    key = (reps, delay_iters)
    if key not in _CACHE:
        _CACHE[key] = _build(reps, delay_iters)
    return _CACHE[key]


def kernel(x, WQ, WK, WV, WO):
    from concourse import bass_utils
    x = np.asarray(x, dtype=np.float32)
    WQ = np.asarray(WQ, dtype=np.float32)
    WK = np.asarray(WK, dtype=np.float32)
    WV = np.asarray(WV, dtype=np.float32)
    WO = np.asarray(WO, dtype=np.float32)
    nc = get_compiled()
    in_maps = _make_in_maps(x, WQ, WK, WV, WO)
    res = bass_utils.run_bass_kernel_spmd(nc, in_maps, core_ids=list(range(N_CORES)))
    out = np.zeros((B, S, D), dtype=np.float32)
    for c in range(N_CORES):
        out[c // HPG] += res.results[c]["out"].astype(np.float32)
    return out
